# revision 19
# baseline (speedup 1.0000x reference)
"""GQA attention kernel for 8 TRN2 NeuronCores.

Sharding (hardcoded): 8 cores = batch(2) x kv-group(4).
Core i handles batch b=i//4, group g=i%4:
  x    = hidden_states[b]                  [2048, 2048] (bf16, host cast)
  wqkv = [Wq | Wk | Wv] group g's columns  [2048, 768]  (bf16, host pack)
  wo   = Wo[g*512:(g+1)*512, :]            [512, 2048]  (bf16, host-
         permuted rows)
  cs   = RoPE cos/sin tables               [128, 16, 2, 64] f32 (host)
Each core returns a partial output [2048, 2048] f32; host sums the 4
group partials per batch.

Per-core pipeline (all matmuls bf16 -> f32 PSUM):
  A) ALL input DMAs go on the single sync HWDGE queue in FIFO order
     (cs, wqkv, 16 XBAR transposes of X into the X^T block layout, wo).
     One queue => program order == completion order; no cross-queue
     DMA-semaphore round-robin stalls.  No SWDGE, no casts on device.
  B+C are ONE woven emission stream (engine queues are strict FIFO, so
     phase-C work can only start early if it is EMITTED early):
     - head: tiles 0-3 get full QKV+RoPE+PE-transpose into QKT
       ([dim, tok]; QT = QKT[:, 0:4, :], KT = QKT[:, 4, :]);
     - (qc0, m0) attention slots are woven with the kv projections of
       tiles 4-15 (2 tiles of lookahead) so KT lands at kv pace and the
       exp chain starts at ~75us instead of after all of phase B;
     - the q projections of tiles 4-15 are spread over later slots,
       always completing a full q-chunk ahead of the qc that reads them;
     - per slot: scores for the kv0 head (PE rows 0-63) and kv1 head
       (rows 64-127) issued back-to-back -> row-tiled matmuls run
       concurrently at full array width; exp is one [128,1024] ACT
       instruction over both PSUM banks (the scalar queue carries
       nothing but exps); PV lags 3 slots ACROSS group boundaries so
       the PE never drains on the group-end normalize; one Wo job
       (4 matmuls + evac + store) every 4th slot from qc1 on.
     V is kept [tok, d] with a ones column per kv head: row 64 of the
     PV psum accumulates the softmax denominators for free.
     PSUM budget (8 banks): scores 2x[128,1024]f32 = 4, o_A+o_B = 2,
     misc rotation (projection psums + transpose staging + Wo accum,
     every tile's producer+consumer chain emitted atomically) = 2.
  D) leftover Wo work drains after the loops, alternating evac between
     scalar/vector and stores between both HWDGE queues.
"""

import math
import numpy as np

S = 2048
HID = 2048
NT = 16          # token tiles of 128
NR = 16          # hid tiles of 128
QD = 512         # q dims per core (8 heads x 64)
KD = 128         # kv dims per core (2 kv heads x 64)
D = 64
NQH = 8          # q heads per core
PI = math.pi

_CACHE = {}


def _build():
    import concourse.bass as bass
    import concourse.mybir as mybir
    from concourse import bacc
    from concourse.tile import TileContext
    from concourse.masks import make_identity

    f32 = mybir.dt.float32
    bf16 = mybir.dt.bfloat16
    AF = mybir.ActivationFunctionType
    OP = mybir.AluOpType

    nc = bacc.Bacc("TRN2", target_bir_lowering=False, debug=False)
    x = nc.dram_tensor("x", [HID, S], bf16, kind="ExternalInput").ap()
    wqkv = nc.dram_tensor("wqkv", [HID, QD + 2 * KD], bf16,
                          kind="ExternalInput").ap()
    wo = nc.dram_tensor("wo", [QD, HID], bf16, kind="ExternalInput").ap()
    cs = nc.dram_tensor("cs", [128, NT, 2, D], f32,
                        kind="ExternalInput").ap()
    out = nc.dram_tensor("out", [S, HID], f32, kind="ExternalOutput").ap()

    with TileContext(nc) as tc:
        with (
            tc.tile_pool(name="const", bufs=1) as const,
            tc.tile_pool(name="wts", bufs=1) as wts,
            tc.tile_pool(name="xt", bufs=1) as xtp,
            tc.tile_pool(name="stage", bufs=2) as stage,
            tc.tile_pool(name="tmps", bufs=3) as tmps,
            tc.tile_pool(name="pbf", bufs=3) as pbf,
            tc.tile_pool(name="rbp", bufs=2) as rbp,
            tc.tile_pool(name="outp", bufs=3) as outp,
        ):
            # ---- input DMA stream, all on the sync HWDGE queue ---------
            cs_sb = const.tile([128, NT, 2, D], f32, tag="cs")
            nc.sync.dma_start(out=cs_sb[:], in_=cs)

            wqkv_sb = wts.tile([128, NR, QD + 2 * KD], bf16, tag="wqkv")
            wo_sb = wts.tile([128, 4, HID], bf16, tag="wo")
            wr = wqkv.rearrange("(r p) q -> p r q", p=128)
            nc.sync.dma_start(out=wqkv_sb[:, 0:8, :], in_=wr[:, 0:8, :])
            nc.sync.dma_start(out=wqkv_sb[:, 8:16, :], in_=wr[:, 8:16, :])

            # X^T hid-major blocks via plain contiguous DMA (x is
            # host-transposed): xr[r][p, c] = X^T[r*128 + p, c].  All of
            # X^T is resident by ~33us, so the woven kv/q projections and
            # the qc0 exp chain are never DMA-feed paced.
            xr = []
            for r in range(NR):
                xr_r = xtp.tile([128, S], bf16, tag="xt", bufs=NR)
                nc.sync.dma_start(out=xr_r[:],
                                  in_=x[r * 128:(r + 1) * 128, :])
                xr.append(xr_r)

            nc.sync.dma_start(
                out=wo_sb[:], in_=wo.rearrange("(d p) n -> p d n", p=128))

            ident = const.tile([128, 128], bf16, tag="ident")
            make_identity(nc, ident[:])

            # outputs of phase B: QKT[dim, blk, tok] with QT = blks 0-3,
            # KT = blk 4
            QKT = wts.tile([128, 5, S], bf16, tag="QKT")
            V = wts.tile([128, NT, 2, 65], bf16, tag="V")  # [tok128,t,kvh,d+1]
            nc.vector.memset(V[:, :, :, 64:65], 1.0)
            attnT = wts.tile([128, 4, S], bf16, tag="attnT")

            # -------- Phases B+C woven: one emission stream ------------
            # PSUM budget (8 banks): sc 2x[128,1024]f32 = 4, o_A+o_B = 2,
            # misc (projections / transpose staging / Wo accum) 2x2KB = 2.
            # Every misc tile's full producer+consumer chain is emitted
            # atomically so the bufs=2 rotation can never deadlock the PE
            # FIFO.
            from collections import deque

            with (
                tc.tile_pool(name="psS", bufs=2, space="PSUM") as psS,
                tc.tile_pool(name="psO", bufs=1, space="PSUM") as psO,
                tc.tile_pool(name="misc", bufs=2, space="PSUM") as misc,
            ):
                def rope(src, dst, n_h, t):
                    # dst = RoPE(src); q heads permuted so head h lands at
                    # col (h%4)*128 + (h//4)*64: after transpose head h
                    # sits at QKT blk h%4, partition half (h//4)*64 == its
                    # kv head's base (wo rows host-permuted to match).
                    cos_t = cs_sb[:, t, 0, :]
                    sin_t = cs_sb[:, t, 1, :]
                    if n_h == NQH:
                        v3 = src.rearrange(
                            "p (half blk d) -> p half blk d", half=2, d=64)
                        o3 = dst.rearrange(
                            "p (blk half d) -> p half blk d", half=2, d=64)
                        sh = [128, 2, 4, 32]
                        c1 = cos_t[:, None, None, 0:32].broadcast_to(sh)
                        s1 = sin_t[:, None, None, 0:32].broadcast_to(sh)
                        c2 = cos_t[:, None, None, 32:64].broadcast_to(sh)
                        s2 = sin_t[:, None, None, 32:64].broadcast_to(sh)
                        q1, q2 = v3[:, :, :, 0:32], v3[:, :, :, 32:64]
                        oa, ob = o3[:, :, :, 0:32], o3[:, :, :, 32:64]
                    else:
                        v3 = src.rearrange("p (h d) -> p h d", d=64)
                        o3 = dst.rearrange("p (h d) -> p h d", d=64)
                        sh = [128, n_h, 32]
                        c1 = cos_t[:, None, 0:32].broadcast_to(sh)
                        s1 = sin_t[:, None, 0:32].broadcast_to(sh)
                        c2 = cos_t[:, None, 32:64].broadcast_to(sh)
                        s2 = sin_t[:, None, 32:64].broadcast_to(sh)
                        q1, q2 = v3[:, :, 0:32], v3[:, :, 32:64]
                        oa, ob = o3[:, :, 0:32], o3[:, :, 32:64]
                    t1 = tmps.tile(sh, f32, tag="t1")
                    t2 = tmps.tile(sh, f32, tag="t2")
                    nc.vector.tensor_tensor(t1[:], q1, c1, OP.mult)
                    nc.vector.tensor_tensor(t2[:], q2, s1, OP.mult)
                    nc.vector.tensor_tensor(oa, t1[:], t2[:], OP.subtract)
                    nc.vector.tensor_tensor(t1[:], q2, c2, OP.mult)
                    nc.vector.tensor_tensor(t2[:], q1, s2, OP.mult)
                    nc.vector.tensor_tensor(ob, t1[:], t2[:], OP.add)

                # kv/q work is split into a "main" half (matmuls + RoPE,
                # emitted ahead) and a "tp" half (PE transpose + DVE evac,
                # emitted a round later so its RoPE wait never head-blocks
                # the PE FIFO).  Both halves keep their psum tile's full
                # producer+consumer chain atomic within the misc rotation.
                kq_stage = {}

                def kv_main(t):
                    ps_kv = misc.tile([128, 2 * KD], f32, tag="m",
                                      name="ps_kv")
                    for r in range(NR):
                        nc.tensor.matmul(
                            ps_kv[:],
                            lhsT=xr[r][:, t * 128:(t + 1) * 128],
                            rhs=wqkv_sb[:, r, QD:QD + 2 * KD],
                            start=(r == 0), stop=(r == NR - 1))
                    kst = stage.tile([128, KD], bf16, tag="qkk")
                    rope(ps_kv[:, 0:KD], kst[:], 2, t)
                    nc.vector.tensor_copy(
                        V[:, t, :, 0:64],
                        ps_kv[:, KD:2 * KD].rearrange(
                            "p (h d) -> p h d", d=64))
                    kq_stage[("k", t)] = kst

                def kv_tp(t):
                    kst = kq_stage.pop(("k", t))
                    tp = misc.tile([128, 128], bf16, tag="m", name="tp_k")
                    nc.tensor.transpose(tp[:], kst[:], ident[:])
                    nc.vector.tensor_copy(
                        QKT[:, 4, t * 128:(t + 1) * 128], tp[:])

                def q_main(t):
                    ps_q = misc.tile([128, QD], f32, tag="m", name="ps_q")
                    for r in range(NR):
                        nc.tensor.matmul(
                            ps_q[:],
                            lhsT=xr[r][:, t * 128:(t + 1) * 128],
                            rhs=wqkv_sb[:, r, 0:QD],
                            start=(r == 0), stop=(r == NR - 1))
                    qst = stage.tile([128, QD], bf16, tag="qkq")
                    rope(ps_q[:, 0:QD], qst[:], NQH, t)
                    kq_stage[("q", t)] = qst

                def q_tp(t):
                    qst = kq_stage.pop(("q", t))
                    tp = misc.tile([128, 4, 128], bf16, tag="m",
                                   name="tp_q")
                    for db in range(4):
                        nc.tensor.transpose(
                            tp[:, db, :], qst[:, db * 128:(db + 1) * 128],
                            ident[:])
                    nc.vector.tensor_copy(
                        QKT[:, 0:4, t * 128:(t + 1) * 128], tp[:])

                def kv_part(t):
                    kv_main(t)
                    kv_tp(t)

                def q_part(t):
                    q_main(t)
                    q_tp(t)

                wo_state = {"t": 0, "nch": 0}

                def wo_job(max_t, alt=False):
                    # one full Wo psum group: 4 matmuls + evac + store.
                    # At drain time (alt=True) alternate the evac between
                    # scalar/vector and the store between the two HWDGE
                    # queues so the tail is not serialized on one engine.
                    st = wo_state
                    if st["t"] >= max_t:
                        return
                    t, nch = st["t"], st["nch"]
                    w_ps = misc.tile([128, 512], f32, tag="m", name="w_ps")
                    for db in range(4):
                        nc.tensor.matmul(
                            w_ps[:],
                            lhsT=attnT[:, db, t * 128:(t + 1) * 128],
                            rhs=wo_sb[:, db, nch * 512:(nch + 1) * 512],
                            start=(db == 0), stop=(db == 3))
                    o_c = outp.tile([128, 512], f32, tag="out", name="o_c")
                    odd = alt and (t * 4 + nch) % 2 == 1
                    if odd:
                        nc.scalar.copy(o_c[:], w_ps[:])
                    else:
                        nc.vector.tensor_copy(o_c[:], w_ps[:])
                    (nc.scalar if odd else nc.sync).dma_start(
                        out=out[t * 128:(t + 1) * 128,
                                nch * 512:(nch + 1) * 512],
                        in_=o_c[:])
                    st["nch"] += 1
                    if st["nch"] == 4:
                        st["nch"] = 0
                        st["t"] += 1

                def new_group(qc, m):
                    o_A = psO.tile([65, 512], f32, tag="oA", name="o_A")
                    o_B = psO.tile([65, 512], f32, tag="oB", name="o_B")
                    return {"qc": qc, "m": m, "o_A": o_A, "o_B": o_B}

                def emit_pv(g, p, kt):
                    nc.tensor.matmul(
                        g["o_A"][:], lhsT=V[:, kt, 0, :], rhs=p[:, 0:512],
                        start=(kt == 0), stop=(kt == NT - 1))
                    nc.tensor.matmul(
                        g["o_B"][:], lhsT=V[:, kt, 1, :],
                        rhs=p[:, 512:1024],
                        start=(kt == 0), stop=(kt == NT - 1))
                    if kt == NT - 1:
                        normalize(g)

                def normalize(g):
                    # row 64 of the PV psum carries the softmax denominator
                    qc, m = g["qc"], g["m"]
                    for (o_ps, qr) in ((g["o_A"], 0), (g["o_B"], 64)):
                        rsum = rbp.tile([1, 512], f32, tag="rsum", bufs=2)
                        nc.vector.tensor_copy(rsum[:], o_ps[64:65, :])
                        recip = rbp.tile([1, 512], f32, tag="recip",
                                         bufs=2)
                        nc.vector.reciprocal_approx_fast(recip[:], rsum[:])
                        rb = rbp.tile([64, 512], f32, tag="rb", bufs=2)
                        nc.gpsimd.partition_broadcast(rb[:], recip[:])
                        nc.vector.tensor_tensor(
                            attnT[qr:qr + 64, m, qc * 512:(qc + 1) * 512],
                            o_ps[0:64, :], rb[:], OP.mult)

                # PV lag runs ACROSS group boundaries (depth 3) so the PE
                # never drains at a boundary and ACT always has 3 score
                # slots of runway over the normalize latency.
                pend = deque()

                def slot(g, kt):
                    qc, m = g["qc"], g["m"]
                    sc = psS.tile([128, 1024], f32, tag="sc")
                    nc.tensor.matmul(
                        sc[:, 0:512],
                        lhsT=QKT[0:64, 4, kt * 128:(kt + 1) * 128],
                        rhs=QKT[0:64, m, qc * 512:(qc + 1) * 512],
                        start=True, stop=True)
                    nc.tensor.matmul(
                        sc[:, 512:1024],
                        lhsT=QKT[64:128, 4, kt * 128:(kt + 1) * 128],
                        rhs=QKT[64:128, m, qc * 512:(qc + 1) * 512],
                        start=True, stop=True)
                    p = pbf.tile([128, 1024], bf16, tag="p", bufs=5)
                    nc.scalar.activation(p[:], sc[:], AF.Exp, scale=0.125)
                    pend.append((g, p, kt))
                    if len(pend) > 3:
                        emit_pv(*pend.popleft())

                def drain_pvs():
                    while pend:
                        emit_pv(*pend.popleft())

                # ---- emission: head tiles 0-3 fully, then qc0-m0 woven
                # with the remaining kv parts (2 tiles of lookahead so
                # scores never head-block the PE FIFO), then qc0 m1-3
                # woven with q parts 4-7 mid-group, then qc1-3 with the
                # remaining q parts and the Wo stream.
                for t in range(4):
                    kv_part(t)
                    q_part(t)
                kv_part(4)
                kv_part(5)
                g = new_group(0, 0)
                for kt in range(4):
                    slot(g, kt)
                for kt in range(4, NT):
                    if kt + 2 < NT:
                        kv_part(kt + 2)
                    slot(g, kt)

                qpl = deque(range(4, NT))
                for m in range(1, 4):
                    g = new_group(0, m)
                    for kt in range(NT):
                        if kt in (5, 11) and len(qpl) > 12 - 4 * m:
                            q_part(qpl.popleft())
                        slot(g, kt)

                for qc in range(1, 4):
                    for m in range(4):
                        g = new_group(qc, m)
                        for kt in range(NT):
                            if kt == 6 and qpl:
                                q_part(qpl.popleft())
                            slot(g, kt)
                            if (m * NT + kt) % 4 == 3:
                                wo_job(qc * 4)
                        if qc == 3:
                            wo_job(12)

                drain_pvs()
                # ---------------- drain remaining Wo -------------------
                while wo_state["t"] < NT:
                    wo_job(NT, alt=True)

    nc.compile()
    return nc


def _get_nc():
    if "nc" not in _CACHE:
        _CACHE["nc"] = _build()
    return _CACHE["nc"]


def _rope_tables():
    p = np.arange(128, dtype=np.float32)[:, None, None]
    t = np.arange(NT, dtype=np.float32)[None, :, None]
    d = np.arange(D)
    inv = (10000.0 ** (-(d % 32).astype(np.float32) / 32.0))[None, None, :]
    ang = (t * 128.0 + p) * inv  # [128, NT, 64]
    return np.ascontiguousarray(
        np.stack([np.cos(ang), np.sin(ang)], axis=2).astype(np.float32))


def _shard(inputs):
    import ml_dtypes
    bf = ml_dtypes.bfloat16
    hs = np.asarray(inputs["hidden_states"], np.float32).astype(bf)
    Wq = np.asarray(inputs["Wq"], np.float32)
    Wk = np.asarray(inputs["Wk"], np.float32)
    Wv = np.asarray(inputs["Wv"], np.float32)
    Wo = np.asarray(inputs["Wo"], np.float32)
    cs = _rope_tables()
    in_maps = []
    for i in range(8):
        b, g = divmod(i, 4)
        wqkv = np.concatenate(
            [Wq[:, g * 512:(g + 1) * 512],
             Wk[:, g * 128:(g + 1) * 128],
             Wv[:, g * 128:(g + 1) * 128]], axis=1).astype(bf)
        in_maps.append({
            "x": np.ascontiguousarray(hs[b].T),
            "wqkv": np.ascontiguousarray(wqkv),
            "wo": np.ascontiguousarray(
                Wo[g * 512:(g + 1) * 512, :].reshape(8, 64, HID)[
                    [0, 4, 1, 5, 2, 6, 3, 7]].reshape(512, HID).astype(bf)),
            "cs": cs,
        })
    return in_maps


def run(inputs, trace=False, tmpdir=None):
    """Run on 8 cores; returns (output [2,2048,2048] f32, exec_time_ns)."""
    from concourse.bass_utils import run_bass_kernel_spmd

    nc = _get_nc()
    in_maps = _shard(inputs)
    kwargs = {}
    if trace:
        import sys, types
        from trn_agent_boot.trn_boot import _ntff_profile_via_ctypes
        if "antenv.axon_hooks" not in sys.modules:
            mod = types.ModuleType("antenv.axon_hooks")
            hook = _ntff_profile_via_ctypes("/opt/axon/libaxon_pjrt.so")
            mod.get_axon_ntff_profile_hook = lambda: hook
            sys.modules["antenv.axon_hooks"] = mod
        import concourse.bass_utils as bu
        bu.upload_artifacts = lambda d: f"local://{d}"
        kwargs = {"trace": True, "tmpdir": tmpdir}
    res = run_bass_kernel_spmd(nc, in_maps, core_ids=list(range(8)), **kwargs)
    full = np.zeros((2, S, HID), np.float32)
    for i in range(8):
        b = i // 4
        full[b] += res.results[i]["out"]
    return full, res.exec_time_ns


def kernel(**inputs):
    out, _ = run(inputs)
    return out


# revision 20
# speedup vs baseline: 1.0317x; 1.0317x over previous
"""GQA attention kernel for 8 TRN2 NeuronCores.

Sharding (hardcoded): 8 cores = batch(2) x kv-group(4).
Core i handles batch b=i//4, group g=i%4:
  x    = hidden_states[b]                  [2048, 2048] (bf16, host cast)
  wqkv = [Wq | Wk | Wv] group g's columns  [2048, 768]  (bf16, host pack)
  wo   = Wo[g*512:(g+1)*512, :]            [512, 2048]  (bf16, host-
         permuted rows)
  cs   = RoPE cos/sin tables               [128, 16, 2, 64] f32 (host)
Each core returns a partial output [2048, 2048] f32; host sums the 4
group partials per batch.

Per-core pipeline (all matmuls bf16 -> f32 PSUM):
  A) ALL input DMAs go on the single sync HWDGE queue in FIFO order
     (cs, wqkv, 16 XBAR transposes of X into the X^T block layout, wo).
     One queue => program order == completion order; no cross-queue
     DMA-semaphore round-robin stalls.  No SWDGE, no casts on device.
  B+C are ONE woven emission stream (engine queues are strict FIFO, so
     phase-C work can only start early if it is EMITTED early):
     - head: tiles 0-3 get full QKV+RoPE+PE-transpose into QKT
       ([dim, tok]; QT = QKT[:, 0:4, :], KT = QKT[:, 4, :]);
     - (qc0, m0) attention slots are woven with the kv projections of
       tiles 4-15 (2 tiles of lookahead) so KT lands at kv pace and the
       exp chain starts at ~75us instead of after all of phase B;
     - the q projections of tiles 4-15 are spread over later slots,
       always completing a full q-chunk ahead of the qc that reads them;
     - per slot: scores for the kv0 head (PE rows 0-63) and kv1 head
       (rows 64-127) issued back-to-back -> row-tiled matmuls run
       concurrently at full array width; exp is one [128,1024] ACT
       instruction over both PSUM banks (the scalar queue carries
       nothing but exps); PV lags 3 slots ACROSS group boundaries so
       the PE never drains on the group-end normalize; one Wo job
       (4 matmuls + evac + store) every 4th slot from qc1 on.
     V is kept [tok, d] with a ones column per kv head: row 64 of the
     PV psum accumulates the softmax denominators for free.
     PSUM budget (8 banks): scores 2x[128,1024]f32 = 4, o_A+o_B = 2,
     misc rotation (projection psums + transpose staging + Wo accum,
     every tile's producer+consumer chain emitted atomically) = 2.
  D) leftover Wo work drains after the loops, alternating evac between
     scalar/vector and stores between both HWDGE queues.
"""

import math
import numpy as np

S = 2048
HID = 2048
NT = 16          # token tiles of 128
NR = 16          # hid tiles of 128
QD = 512         # q dims per core (8 heads x 64)
KD = 128         # kv dims per core (2 kv heads x 64)
D = 64
NQH = 8          # q heads per core
PI = math.pi

_CACHE = {}


def _build():
    import concourse.bass as bass
    import concourse.mybir as mybir
    from concourse import bacc
    from concourse.tile import TileContext
    from concourse.masks import make_identity

    f32 = mybir.dt.float32
    bf16 = mybir.dt.bfloat16
    AF = mybir.ActivationFunctionType
    OP = mybir.AluOpType

    nc = bacc.Bacc("TRN2", target_bir_lowering=False, debug=False)
    x = nc.dram_tensor("x", [S, HID], bf16, kind="ExternalInput").ap()
    wqkv = nc.dram_tensor("wqkv", [HID, QD + 2 * KD], bf16,
                          kind="ExternalInput").ap()
    wo = nc.dram_tensor("wo", [QD, HID], bf16, kind="ExternalInput").ap()
    cs = nc.dram_tensor("cs", [128, NT, 2, D], f32,
                        kind="ExternalInput").ap()
    out = nc.dram_tensor("out", [S, HID], f32, kind="ExternalOutput").ap()

    with TileContext(nc) as tc:
        with (
            tc.tile_pool(name="const", bufs=1) as const,
            tc.tile_pool(name="wts", bufs=1) as wts,
            tc.tile_pool(name="xt", bufs=1) as xtp,
            tc.tile_pool(name="stage", bufs=2) as stage,
            tc.tile_pool(name="tmps", bufs=3) as tmps,
            tc.tile_pool(name="pbf", bufs=3) as pbf,
            tc.tile_pool(name="rbp", bufs=2) as rbp,
            tc.tile_pool(name="outp", bufs=3) as outp,
        ):
            # ---- input DMA stream, all on the sync HWDGE queue ---------
            cs_sb = const.tile([128, NT, 2, D], f32, tag="cs")
            nc.sync.dma_start(out=cs_sb[:], in_=cs)

            wqkv_sb = wts.tile([128, NR, QD + 2 * KD], bf16, tag="wqkv")
            wo_sb = wts.tile([128, 4, HID], bf16, tag="wo")
            wr = wqkv.rearrange("(r p) q -> p r q", p=128)
            nc.sync.dma_start(out=wqkv_sb[:, 0:8, :], in_=wr[:, 0:8, :])
            nc.sync.dma_start(out=wqkv_sb[:, 8:16, :], in_=wr[:, 8:16, :])

            # X^T blocks: xt[t][p, r, c] = x[t*128 + c, r*128 + p]
            xt = []
            for t in range(NT):
                xt_t = xtp.tile([128, NR, 128], bf16, tag="xt", bufs=NT)
                nc.sync.dma_start(out=xt_t[:],
                                  in_=x[t * 128:(t + 1) * 128, :],
                                  transpose=True)
                xt.append(xt_t)

            nc.sync.dma_start(
                out=wo_sb[:], in_=wo.rearrange("(d p) n -> p d n", p=128))

            ident = const.tile([128, 128], bf16, tag="ident")
            make_identity(nc, ident[:])

            # outputs of phase B: QKT[dim, blk, tok] with QT = blks 0-3,
            # KT = blk 4
            QKT = wts.tile([128, 5, S], bf16, tag="QKT")
            V = wts.tile([128, NT, 2, 65], bf16, tag="V")  # [tok128,t,kvh,d+1]
            nc.vector.memset(V[:, :, :, 64:65], 1.0)
            attnT = wts.tile([128, 4, S], bf16, tag="attnT")

            # -------- Phases B+C woven: one emission stream ------------
            # PSUM budget (8 banks): sc 2x[128,1024]f32 = 4, o_A+o_B = 2,
            # misc (projections / transpose staging / Wo accum) 2x2KB = 2.
            # Every misc tile's full producer+consumer chain is emitted
            # atomically so the bufs=2 rotation can never deadlock the PE
            # FIFO.
            from collections import deque

            with (
                tc.tile_pool(name="psS", bufs=2, space="PSUM") as psS,
                tc.tile_pool(name="psO", bufs=1, space="PSUM") as psO,
                tc.tile_pool(name="misc", bufs=2, space="PSUM") as misc,
            ):
                def rope(src, dst, n_h, t):
                    # dst = RoPE(src); q heads permuted so head h lands at
                    # col (h%4)*128 + (h//4)*64: after transpose head h
                    # sits at QKT blk h%4, partition half (h//4)*64 == its
                    # kv head's base (wo rows host-permuted to match).
                    cos_t = cs_sb[:, t, 0, :]
                    sin_t = cs_sb[:, t, 1, :]
                    if n_h == NQH:
                        v3 = src.rearrange(
                            "p (half blk d) -> p half blk d", half=2, d=64)
                        o3 = dst.rearrange(
                            "p (blk half d) -> p half blk d", half=2, d=64)
                        sh = [128, 2, 4, 32]
                        c1 = cos_t[:, None, None, 0:32].broadcast_to(sh)
                        s1 = sin_t[:, None, None, 0:32].broadcast_to(sh)
                        c2 = cos_t[:, None, None, 32:64].broadcast_to(sh)
                        s2 = sin_t[:, None, None, 32:64].broadcast_to(sh)
                        q1, q2 = v3[:, :, :, 0:32], v3[:, :, :, 32:64]
                        oa, ob = o3[:, :, :, 0:32], o3[:, :, :, 32:64]
                    else:
                        v3 = src.rearrange("p (h d) -> p h d", d=64)
                        o3 = dst.rearrange("p (h d) -> p h d", d=64)
                        sh = [128, n_h, 32]
                        c1 = cos_t[:, None, 0:32].broadcast_to(sh)
                        s1 = sin_t[:, None, 0:32].broadcast_to(sh)
                        c2 = cos_t[:, None, 32:64].broadcast_to(sh)
                        s2 = sin_t[:, None, 32:64].broadcast_to(sh)
                        q1, q2 = v3[:, :, 0:32], v3[:, :, 32:64]
                        oa, ob = o3[:, :, 0:32], o3[:, :, 32:64]
                    t1 = tmps.tile(sh, f32, tag="t1")
                    t2 = tmps.tile(sh, f32, tag="t2")
                    nc.vector.tensor_tensor(t1[:], q1, c1, OP.mult)
                    nc.vector.tensor_tensor(t2[:], q2, s1, OP.mult)
                    nc.vector.tensor_tensor(oa, t1[:], t2[:], OP.subtract)
                    nc.vector.tensor_tensor(t1[:], q2, c2, OP.mult)
                    nc.vector.tensor_tensor(t2[:], q1, s2, OP.mult)
                    nc.vector.tensor_tensor(ob, t1[:], t2[:], OP.add)

                # kv/q work is split into a "main" half (matmuls + RoPE,
                # emitted ahead) and a "tp" half (PE transpose + DVE evac,
                # emitted a round later so its RoPE wait never head-blocks
                # the PE FIFO).  Both halves keep their psum tile's full
                # producer+consumer chain atomic within the misc rotation.
                kq_stage = {}

                def kv_main(t):
                    ps_kv = misc.tile([128, 2 * KD], f32, tag="m",
                                      name="ps_kv")
                    for r in range(NR):
                        nc.tensor.matmul(
                            ps_kv[:], lhsT=xt[t][:, r, :],
                            rhs=wqkv_sb[:, r, QD:QD + 2 * KD],
                            start=(r == 0), stop=(r == NR - 1))
                    kst = stage.tile([128, KD], bf16, tag="qkk")
                    rope(ps_kv[:, 0:KD], kst[:], 2, t)
                    nc.vector.tensor_copy(
                        V[:, t, :, 0:64],
                        ps_kv[:, KD:2 * KD].rearrange(
                            "p (h d) -> p h d", d=64))
                    kq_stage[("k", t)] = kst

                def kv_tp(t):
                    kst = kq_stage.pop(("k", t))
                    tp = misc.tile([128, 128], bf16, tag="m", name="tp_k")
                    nc.tensor.transpose(tp[:], kst[:], ident[:])
                    nc.vector.tensor_copy(
                        QKT[:, 4, t * 128:(t + 1) * 128], tp[:])

                def q_main(t):
                    ps_q = misc.tile([128, QD], f32, tag="m", name="ps_q")
                    for r in range(NR):
                        nc.tensor.matmul(
                            ps_q[:], lhsT=xt[t][:, r, :],
                            rhs=wqkv_sb[:, r, 0:QD],
                            start=(r == 0), stop=(r == NR - 1))
                    qst = stage.tile([128, QD], bf16, tag="qkq")
                    rope(ps_q[:, 0:QD], qst[:], NQH, t)
                    kq_stage[("q", t)] = qst

                def q_tp(t):
                    qst = kq_stage.pop(("q", t))
                    tp = misc.tile([128, 4, 128], bf16, tag="m",
                                   name="tp_q")
                    for db in range(4):
                        nc.tensor.transpose(
                            tp[:, db, :], qst[:, db * 128:(db + 1) * 128],
                            ident[:])
                    nc.vector.tensor_copy(
                        QKT[:, 0:4, t * 128:(t + 1) * 128], tp[:])

                def kv_part(t):
                    kv_main(t)
                    kv_tp(t)

                def q_part(t):
                    q_main(t)
                    q_tp(t)

                wo_state = {"t": 0, "nch": 0}

                def wo_job(max_t, alt=False):
                    # one full Wo psum group: 4 matmuls + evac + store.
                    # At drain time (alt=True) alternate the evac between
                    # scalar/vector and the store between the two HWDGE
                    # queues so the tail is not serialized on one engine.
                    st = wo_state
                    if st["t"] >= max_t:
                        return
                    t, nch = st["t"], st["nch"]
                    w_ps = misc.tile([128, 512], f32, tag="m", name="w_ps")
                    for db in range(4):
                        nc.tensor.matmul(
                            w_ps[:],
                            lhsT=attnT[:, db, t * 128:(t + 1) * 128],
                            rhs=wo_sb[:, db, nch * 512:(nch + 1) * 512],
                            start=(db == 0), stop=(db == 3))
                    o_c = outp.tile([128, 512], f32, tag="out", name="o_c")
                    odd = alt and (t * 4 + nch) % 2 == 1
                    if odd:
                        nc.scalar.copy(o_c[:], w_ps[:])
                    else:
                        nc.vector.tensor_copy(o_c[:], w_ps[:])
                    (nc.scalar if odd else nc.sync).dma_start(
                        out=out[t * 128:(t + 1) * 128,
                                nch * 512:(nch + 1) * 512],
                        in_=o_c[:])
                    st["nch"] += 1
                    if st["nch"] == 4:
                        st["nch"] = 0
                        st["t"] += 1

                def new_group(qc, m):
                    o_A = psO.tile([65, 512], f32, tag="oA", name="o_A")
                    o_B = psO.tile([65, 512], f32, tag="oB", name="o_B")
                    return {"qc": qc, "m": m, "o_A": o_A, "o_B": o_B}

                def emit_pv(g, p, kt):
                    nc.tensor.matmul(
                        g["o_A"][:], lhsT=V[:, kt, 0, :], rhs=p[:, 0:512],
                        start=(kt == 0), stop=(kt == NT - 1))
                    nc.tensor.matmul(
                        g["o_B"][:], lhsT=V[:, kt, 1, :],
                        rhs=p[:, 512:1024],
                        start=(kt == 0), stop=(kt == NT - 1))
                    if kt == NT - 1:
                        normalize(g)

                def normalize(g):
                    # row 64 of the PV psum carries the softmax denominator
                    qc, m = g["qc"], g["m"]
                    for (o_ps, qr) in ((g["o_A"], 0), (g["o_B"], 64)):
                        rsum = rbp.tile([1, 512], f32, tag="rsum", bufs=2)
                        nc.vector.tensor_copy(rsum[:], o_ps[64:65, :])
                        recip = rbp.tile([1, 512], f32, tag="recip",
                                         bufs=2)
                        nc.vector.reciprocal_approx_fast(recip[:], rsum[:])
                        rb = rbp.tile([64, 512], f32, tag="rb", bufs=2)
                        nc.gpsimd.partition_broadcast(rb[:], recip[:])
                        nc.vector.tensor_tensor(
                            attnT[qr:qr + 64, m, qc * 512:(qc + 1) * 512],
                            o_ps[0:64, :], rb[:], OP.mult)

                # PV lag runs ACROSS group boundaries (depth 3) so the PE
                # never drains at a boundary and ACT always has 3 score
                # slots of runway over the normalize latency.
                pend = deque()

                def slot(g, kt):
                    qc, m = g["qc"], g["m"]
                    sc = psS.tile([128, 1024], f32, tag="sc")
                    nc.tensor.matmul(
                        sc[:, 0:512],
                        lhsT=QKT[0:64, 4, kt * 128:(kt + 1) * 128],
                        rhs=QKT[0:64, m, qc * 512:(qc + 1) * 512],
                        start=True, stop=True)
                    nc.tensor.matmul(
                        sc[:, 512:1024],
                        lhsT=QKT[64:128, 4, kt * 128:(kt + 1) * 128],
                        rhs=QKT[64:128, m, qc * 512:(qc + 1) * 512],
                        start=True, stop=True)
                    p = pbf.tile([128, 1024], bf16, tag="p", bufs=5)
                    nc.scalar.activation(p[:], sc[:], AF.Exp, scale=0.125)
                    pend.append((g, p, kt))
                    if len(pend) > 3:
                        emit_pv(*pend.popleft())

                def drain_pvs():
                    while pend:
                        emit_pv(*pend.popleft())

                # ---- emission: head tiles 0-3 fully, then qc0-m0 woven
                # with the remaining kv parts (2 tiles of lookahead so
                # scores never head-block the PE FIFO), then qc0 m1-3
                # woven with q parts 4-7 mid-group, then qc1-3 with the
                # remaining q parts and the Wo stream.
                for t in range(4):
                    kv_part(t)
                    q_part(t)
                kv_part(4)
                kv_part(5)
                g = new_group(0, 0)
                for kt in range(4):
                    slot(g, kt)
                for kt in range(4, NT):
                    if kt + 2 < NT:
                        kv_part(kt + 2)
                    slot(g, kt)

                qpl = deque(range(4, NT))
                for m in range(1, 4):
                    g = new_group(0, m)
                    for kt in range(NT):
                        if kt in (5, 11) and len(qpl) > 12 - 4 * m:
                            q_part(qpl.popleft())
                        slot(g, kt)

                for qc in range(1, 4):
                    for m in range(4):
                        g = new_group(qc, m)
                        for kt in range(NT):
                            if kt == 6 and qpl:
                                q_part(qpl.popleft())
                            slot(g, kt)
                            if (m * NT + kt) % 4 == 3:
                                wo_job(qc * 4)
                        if qc == 3:
                            wo_job(12)

                drain_pvs()
                # ---------------- drain remaining Wo -------------------
                while wo_state["t"] < NT:
                    wo_job(NT, alt=True)

    nc.compile()
    return nc


def _get_nc():
    if "nc" not in _CACHE:
        _CACHE["nc"] = _build()
    return _CACHE["nc"]


def _rope_tables():
    p = np.arange(128, dtype=np.float32)[:, None, None]
    t = np.arange(NT, dtype=np.float32)[None, :, None]
    d = np.arange(D)
    inv = (10000.0 ** (-(d % 32).astype(np.float32) / 32.0))[None, None, :]
    ang = (t * 128.0 + p) * inv  # [128, NT, 64]
    return np.ascontiguousarray(
        np.stack([np.cos(ang), np.sin(ang)], axis=2).astype(np.float32))


def _shard(inputs):
    import ml_dtypes
    bf = ml_dtypes.bfloat16
    hs = np.asarray(inputs["hidden_states"], np.float32).astype(bf)
    Wq = np.asarray(inputs["Wq"], np.float32)
    Wk = np.asarray(inputs["Wk"], np.float32)
    Wv = np.asarray(inputs["Wv"], np.float32)
    Wo = np.asarray(inputs["Wo"], np.float32)
    cs = _rope_tables()
    in_maps = []
    for i in range(8):
        b, g = divmod(i, 4)
        wqkv = np.concatenate(
            [Wq[:, g * 512:(g + 1) * 512],
             Wk[:, g * 128:(g + 1) * 128],
             Wv[:, g * 128:(g + 1) * 128]], axis=1).astype(bf)
        in_maps.append({
            "x": np.ascontiguousarray(hs[b]),
            "wqkv": np.ascontiguousarray(wqkv),
            "wo": np.ascontiguousarray(
                Wo[g * 512:(g + 1) * 512, :].reshape(8, 64, HID)[
                    [0, 4, 1, 5, 2, 6, 3, 7]].reshape(512, HID).astype(bf)),
            "cs": cs,
        })
    return in_maps


def run(inputs, trace=False, tmpdir=None):
    """Run on 8 cores; returns (output [2,2048,2048] f32, exec_time_ns)."""
    from concourse.bass_utils import run_bass_kernel_spmd

    nc = _get_nc()
    in_maps = _shard(inputs)
    kwargs = {}
    if trace:
        import sys, types
        from trn_agent_boot.trn_boot import _ntff_profile_via_ctypes
        if "antenv.axon_hooks" not in sys.modules:
            mod = types.ModuleType("antenv.axon_hooks")
            hook = _ntff_profile_via_ctypes("/opt/axon/libaxon_pjrt.so")
            mod.get_axon_ntff_profile_hook = lambda: hook
            sys.modules["antenv.axon_hooks"] = mod
        import concourse.bass_utils as bu
        bu.upload_artifacts = lambda d: f"local://{d}"
        kwargs = {"trace": True, "tmpdir": tmpdir}
    res = run_bass_kernel_spmd(nc, in_maps, core_ids=list(range(8)), **kwargs)
    full = np.zeros((2, S, HID), np.float32)
    for i in range(8):
        b = i // 4
        full[b] += res.results[i]["out"]
    return full, res.exec_time_ns


def kernel(**inputs):
    out, _ = run(inputs)
    return out
